# revision 64
# baseline (speedup 1.0000x reference)
"""Multi-head attention (B=4, N=2048, C=768, H=12, Dh=64) on 8 TRN2 NeuronCores.

Sharding: head-parallel within each batch. Core (b, g) (g = core % 2) computes
q/k/v for heads 6g..6g+5 of batch b over the FULL 2048-token sequence -- no
duplicated K/V work between the pair -- runs attention for its 6 heads x 2048
queries, and produces a PARTIAL projection over its 384 channels. The host
sums the two partial (bf16) [2048, 768] outputs per batch (host-side
all-reduce; no device collectives). The bias rides in core g=0's input; core
g=1 receives zeros, keeping the program SPMD-identical.

Per-core inputs (partition dim first; all host-repacked so every priority
DMA descriptor reads contiguous DRAM):
  pri    [6, 128, 1152] bf16  per 128-row chunk: [x cols 0:512 | q0|k0 | v]
                              -- the complete dependency set of the first
                              q/k chains and all v chains
  xrest  [3, 6, 128, 512] bf16  x token-columns 512:2048 in (colchunk, cc)
  wqrest [768, 512]  bf16  [q1|k1|q2|k2] weight columns
  wprojT [384, 768]  bf16  proj_w.T rows for this core's 384 channels
  bias   [1, 768]    f32   real bias for g=0, zeros for g=1
  out    [2048, 768] bf16  partial projection (bf16 halves the drain)

Pipeline (~266us measured; evolved 371 -> 275 -> 266 via trace analysis;
TensorE ~99% occupied through the middle, ScalarE exp = 214us busy floor):
  - 12 attention blocks (3 local head pairs x 4 query i-blocks) in PAIR-MAJOR
    order so each pair's q/k chains are produced as filler inside the
    previous pair's exp-bound blocks.
  - per kv chunk: the two heads' score matmuls (K=64, lhsT base partitions
    0/64 -> auto row-tiled, they run CONCURRENTLY in the PE array) write a
    [128, 1024] PSUM tile; ONE 1024-wide exp per chunk on ScalarE (scale
    folds 1/sqrt(Dh); no max-subtraction, |S| <= ~10).
  - scores are emitted in adjacent-j pairs and the next block's first TWO
    score groups are emitted before this block's trailing PVs, so the exp
    stream crosses block boundaries without a bubble (the ~1.2us/transition
    ACT gaps this removes were worth ~13us).
  - the PV pair runs TWO steps behind its exp; PV stationary is a 128-col
    window into the packed V tile [v_h|1|v_{h+1}|1|...]; psum row 64
    accumulates the softmax denominator, rows 65..127 are garbage.
  - q/k/v/proj matmul chains are emitted as "filler" ops drained into the
    exp-bound steps at a per-block rate; every chain fully emits before the
    block that reads it. Blocks (0,0)-(3,0) are PE-capacity-bound (the 16 v
    chains must materialize inside block (0,0)); the ~12us of ACT idle there
    is the irreducible cost of this sharding.
  - input DMA: a dma trigger BLOCKS its engine queue until the transfer
    completes, so the scalar(ACT) ring carries exactly two early pri pieces
    and nothing else; priority pieces split across sync/scalar/gpsimd rings
    (~83GB/s each), bulk x columns ordered by consumption time. First real
    exp starts ~17-21us in (vs ~29us before), of which 7.5us is fixed NEFF
    startup.
  - a ~5us burst of tiny matmuls at t=7us flips the PE HAM clock gate to
    2.4GHz before the first chains run (without it they run at 1.2GHz).
    CAUTION: a longer burst (80 matmuls) tripped a chip-wide ~1.2x
    power-state throttle for the entire run -- keep the burst ~26 matmuls.
  - normalize: denominator rows copied to SBUF (reciprocal_approx_fast
    reading PSUM directly returned garbage -- keep the copy), per-head
    reciprocal, GpSimd partition broadcast, multiply; pv psum copied out so
    the next block's accumulation is released quickly (last block reads psum
    direct and drains PVs eagerly, pvdepth=1).
  - tail: proj for the last 4 output rows borrows the st psum banks (free
    after the last exp) so consecutive row-blocks double-buffer instead of
    serializing on the 2-bank qk pool; their outputs drain as split DMAs
    across three rings.
"""

import sys

if "/opt/trn_rl_repo" not in sys.path:
    sys.path.insert(0, "/opt/trn_rl_repo")

import numpy as np
import ml_dtypes

B, N, C = 4, 2048, 768
H, Dh = 12, 64
HL = 6             # heads per core
CL = HL * Dh       # 384 local channels
SCALE = Dh ** -0.5
CCH = C // 128     # 6 contraction chunks (x/qkv input dim)
CCL = CL // 128    # 3 local head-pair chunks
NCORES = 8
VW = (HL - 1) * 65 + 128  # padded width of packed v tiles (453)
WARMUP = True

_NC_CACHE = {}


def _build():
    import concourse.bass as bass
    import concourse.tile as tile
    import concourse.mybir as mybir
    from concourse import bacc

    f32 = mybir.dt.float32
    bf16 = mybir.dt.bfloat16
    Exp = mybir.ActivationFunctionType.Exp

    nc = bacc.Bacc(
        "TRN2",
        target_bir_lowering=False,
        debug=False,
        enable_asserts=False,
        num_devices=NCORES,
    )

    # pri: per 128-row chunk, [x cols 0:512 | q0|k0 weights] packed
    # contiguously on the host so one descriptor covers the first q/k
    # chains' dependencies.  xrest: x cols 512:2048 in (colchunk, cc)
    # pieces, each contiguous.  wqrest: packed [q1|k1|q2|k2|v] columns.
    pri_d = nc.dram_tensor("pri", [CCH, 128, 1152], bf16,
                           kind="ExternalInput").ap()
    xrest_d = nc.dram_tensor("xrest", [3, CCH, 128, 512], bf16,
                             kind="ExternalInput").ap()
    wqrest = nc.dram_tensor("wqrest", [C, 512], bf16,
                            kind="ExternalInput").ap()
    wprojT = nc.dram_tensor("wprojT", [CL, C], bf16, kind="ExternalInput").ap()
    bias = nc.dram_tensor("bias", [1, C], f32, kind="ExternalInput").ap()
    out = nc.dram_tensor("out", [N, C], bf16, kind="ExternalOutput").ap()

    with tile.TileContext(nc) as tc:
        from contextlib import ExitStack

        with ExitStack() as ctx:
            singles = ctx.enter_context(tc.tile_pool(name="singles", bufs=1))
            psum = ctx.enter_context(tc.tile_pool(name="psum", bufs=1, space="PSUM"))
            work = ctx.enter_context(tc.tile_pool(name="work", bufs=4))

            # warm the ACT exp table set while the input DMAs run
            dummy = singles.tile([1, 8], f32, tag="dummy", name="dummy")
            nc.vector.memset(dummy, 0.0)
            dummy2 = singles.tile([1, 8], f32, tag="dummy2", name="dummy2")
            nc.scalar.activation(dummy2, dummy, Exp)

            # warm the PE HAM clock gate during the input-DMA window: ~3us
            # of back-to-back tiny matmuls flips the PE to 2.4GHz before the
            # first real chains run (they'd otherwise run at 1.2GHz)
            if WARMUP:
                # FULL-WIDTH (K=128, M=128) matmuls: quadrant-sized warmup
                # tiles never tripped the HAM busy detector (chains stayed at
                # 1.2GHz).  memset on gpsimd so the warmup isn't queued
                # behind the DVE's vt-ones memsets.
                warm = singles.tile([128, 224], bf16, tag="warm", name="warm")
                nc.gpsimd.memset(warm, 0.0)
                wps = psum.tile([128, 1024], f32, tag="st", bufs=2,
                                name="warmps")
                for i in range(38):
                    nc.tensor.matmul(wps[:, 0:224], lhsT=warm[:, 0:128],
                                     rhs=warm, start=(i == 0), stop=(i == 37))

            # ---- input DMAs --------------------------------------------
            # The first scores+exp need pri[cc] = [x cols 0:512 | q0|k0
            # weights]: 6 contiguous descriptors split over the sync and
            # scalar rings (~3 each), landing ~10.5us in (7.5us of that is
            # fixed NEFF startup).  The x-column tail streams in 512-col
            # pieces ordered by consumption time; the scalar (ACT) ring
            # stops issuing before the exp stream begins.
            load = tc.alloc_tile_pool(name="load", bufs=1)
            pri = [load.tile([128, 1152], bf16, tag=f"pri{i}", name=f"pri{i}")
                   for i in range(CCH)]
            xt = [load.tile([128, 1536], bf16, tag=f"xt{i}", name=f"xt{i}")
                  for i in range(CCH)]
            wqb = [load.tile([128, 512], bf16, tag=f"wqb{i}", name=f"wqb{i}")
                   for i in range(CCH)]
            wqv = [pri[i][:, 768:1152] for i in range(CCH)]
            # NOTE: a dma trigger BLOCKS its engine queue until the transfer
            # completes (ring flow control), so the scalar(ACT) ring gets
            # exactly ONE early piece -- anything more would sit in front of
            # the first exps.  Everything else rides sync/gpsimd.
            nc.sync.dma_start(out=pri[0], in_=pri_d[0])
            nc.scalar.dma_start(out=pri[2], in_=pri_d[2])
            nc.gpsimd.dma_start(out=pri[3], in_=pri_d[3])
            nc.sync.dma_start(out=pri[1], in_=pri_d[1])
            nc.scalar.dma_start(out=pri[4], in_=pri_d[4])
            nc.gpsimd.dma_start(out=pri[5], in_=pri_d[5])
            for i in range(CCH):
                nc.sync.dma_start(out=xt[i][:, 0:512], in_=xrest_d[0, i])
            for i in range(CCH):
                nc.gpsimd.dma_start(out=xt[i][:, 512:1024], in_=xrest_d[1, i])
            for i in range(3):
                nc.sync.dma_start(out=xt[i][:, 1024:1536], in_=xrest_d[2, i])
                nc.gpsimd.dma_start(out=xt[i + 3][:, 1024:1536],
                                    in_=xrest_d[2, i + 3])
            for i in range(CCH):
                nc.sync.dma_start(out=wqb[i],
                                  in_=wqrest[i * 128:(i + 1) * 128, 0:512])
            wp = []
            for i in range(CCL):
                t = singles.tile([128, C], bf16, tag=f"wp{i}", name=f"wp{i}")
                nc.sync.dma_start(out=t, in_=wprojT[i * 128:(i + 1) * 128, :])
                wp.append(t)
            bias_bc = singles.tile([128, C], f32, tag="bias", name="bias_bc")
            nc.sync.dma_start(
                out=bias_bc,
                in_=bass.AP(tensor=bias.tensor, offset=bias.offset,
                            ap=[[0, 128]] + list(bias.ap[1:])),
            )

            # x token-columns c0:c0+512 of row-chunk cc (pri holds 0:512)
            def xcol(cc, c0, w=512):
                if c0 + w <= 512:
                    return pri[cc][:, c0:c0 + w]
                return xt[cc][:, c0 - 512:c0 - 512 + w]

            # wq column lookup in the packed [q0|k0 || q1|k1|q2|k2 | v] layout
            def wq_qk(cc, base, hp):
                is_k = 1 if base else 0
                if hp == 0:
                    return pri[cc][:, 512 + is_k * 128:512 + (is_k + 1) * 128]
                c0 = (hp - 1) * 256 + is_k * 128
                return wqb[cc][:, c0:c0 + 128]

            # ---- qkv storage -------------------------------------------
            qt = [singles.tile([128, N], bf16, tag=f"qt{i}", name=f"qt{i}")
                  for i in range(CCL)]
            kt = [singles.tile([128, N], bf16, tag=f"kt{i}", name=f"kt{i}")
                  for i in range(CCL)]
            vt = [singles.tile([128, VW], bf16, tag=f"vt{i}", name=f"vt{i}")
                  for i in range(N // 128)]
            att = [singles.tile([128, N], bf16, tag=f"att{i}", name=f"att{i}")
                   for i in range(CCL)]

            # one q-or-k chain: 6 accumulating matmuls + a copy-out.
            # `order` lets the prefix chains accumulate in DMA-arrival order;
            # `split_copy` copies the first 128 columns separately so the
            # first score matmul (which only needs k cols 0:128) starts early.
            def emit_chain(dst, base, hp, nch, order=None, split_copy=False):
                ops = []
                ps = psum.tile([128, 512], f32, tag="qk", bufs=2,
                               name=f"ch{base}{hp}{nch}")
                ccs = list(order) if order else list(range(CCH))
                for i, cc in enumerate(ccs):
                    ops.append(lambda base=base, hp=hp, nch=nch, cc=cc, ps=ps,
                               st=(i == 0), sp=(i == CCH - 1): nc.tensor.matmul(
                        ps,
                        lhsT=wq_qk(cc, base, hp),
                        rhs=xcol(cc, nch * 512),
                        start=st, stop=sp,
                    ))
                c0 = nch * 512
                if split_copy:
                    ops.append(lambda: nc.vector.tensor_copy(
                        dst[hp][:, c0:c0 + 128], ps[:, 0:128]))
                    ops.append(lambda: nc.vector.tensor_copy(
                        dst[hp][:, c0 + 128:c0 + 512], ps[:, 128:512]))
                else:
                    ops.append(lambda: nc.vector.tensor_copy(
                        dst[hp][:, c0:c0 + 512], ps))
                return ops

            def q_ch(hp, n, order=None):
                return emit_chain(qt, 0, hp, n, order)

            def k_ch(hp, n, order=None):
                return emit_chain(kt, CL, hp, n, order)

            # partial proj for one 128-row output block (6 matmuls + add + dma).
            # Tail ics (>=12) borrow the st psum banks (free after the last
            # exp) so consecutive ics double-buffer instead of serializing on
            # the 2-bank qk pool, and their outputs drain as split DMAs on
            # three rings.
            def emit_proj(ic):
                ops = []
                tail = ic >= 12
                if tail and ic % 2 == 0:
                    t = psum.tile([128, 1024], f32, tag="st", bufs=2,
                                  name=f"pjt{ic}")
                    pjs = [(t[:, 0:512], 0, 512), (t[:, 512:768], 512, 256)]
                else:
                    pjs = [(psum.tile([128, 512], f32, tag="qk", bufs=2,
                                      name=f"pj{ic}_{d0}")[:, 0:dw], d0, dw)
                           for (d0, dw) in ((0, 512), (512, 256))]
                for cc in range(CCL):
                    for (pj, d0, dw) in pjs:
                        ops.append(lambda ic=ic, d0=d0, dw=dw, cc=cc, pj=pj: nc.tensor.matmul(
                            pj,
                            lhsT=att[cc][:, ic * 128:(ic + 1) * 128],
                            rhs=wp[cc][:, d0:d0 + dw],
                            start=(cc == 0), stop=(cc == CCL - 1),
                        ))
                def fin(ic=ic, pjs=pjs, tail=tail):
                    osb = work.tile([128, C], bf16, tag="osb", bufs=3,
                                    name=f"osb{ic}")
                    for (pj, d0, dw) in pjs:
                        nc.vector.tensor_add(osb[:, d0:d0 + dw], pj,
                                             bias_bc[:, d0:d0 + dw])
                    qs = [nc.sync, nc.gpsimd, nc.scalar]
                    if tail:
                        qa = qs[ic % 3]
                        qb = qs[(ic + 1) % 3]
                        qa.dma_start(out=out[ic * 128:ic * 128 + 64, :],
                                     in_=osb[0:64, :])
                        qb.dma_start(out=out[ic * 128 + 64:(ic + 1) * 128, :],
                                     in_=osb[64:128, :])
                    else:
                        qs[ic % 2].dma_start(out=out[ic * 128:(ic + 1) * 128, :],
                                             in_=osb)
                ops.append(fin)
                return ops

            # v in [token, d] layout, packed [v_h(64)|1] x 6 heads + pad.
            def emit_vt(nt):
                ops = []
                vaug = vt[nt][:, 0:HL * 65].rearrange("p (h e) -> p h e", e=65)
                ops.append(lambda vaug=vaug: nc.vector.memset(
                    vaug[:, :, 64:65], 1.0))
                ops.append(lambda nt=nt: nc.vector.memset(
                    vt[nt][:, HL * 65:VW], 0.0))
                ps = psum.tile([128, 512], f32, tag="qk", bufs=2,
                               name=f"psv{nt}")
                for cc in range(CCH):
                    ops.append(lambda nt=nt, cc=cc, ps=ps: nc.tensor.matmul(
                        ps[:, 0:CL],
                        lhsT=xcol(cc, nt * 128, 128),
                        rhs=wqv[cc],
                        start=(cc == 0), stop=(cc == CCH - 1),
                    ))
                ops.append(lambda vaug=vaug, ps=ps: nc.vector.tensor_copy(
                    vaug[:, :, 0:64],
                    ps[:, 0:CL].rearrange("p (h e) -> p h e", e=64),
                ))
                return ops

            # ---- phase 0: minimal prefix -------------------------------
            # accumulate in DMA-arrival order (pri pieces land one per ring);
            # v0 is NOT here -- its weights land later and would block the
            # first scores in the PE queue (it leads the block-0 weave)
            ARR = [2, 0, 3, 1, 5, 4]
            for op in q_ch(0, 0, order=ARR):
                op()
            for op in emit_chain(kt, CL, 0, 0, order=ARR, split_copy=True):
                op()

            # ---- attention ---------------------------------------------
            # per-block filler: matmul-ish ops interleaved into exp-bound
            # steps so the PE never idles while ScalarE runs exp.
            NJ = N // 128                     # 16 kv chunks
            filler = []
            # pair-major block order: each head pair's q/k chains are
            # produced inside the PREVIOUS pair's exp-bound blocks.
            fill_plan = {
                (1, 0): lambda: q_ch(0, 2) + q_ch(1, 0) + k_ch(1, 0),
                (2, 0): lambda: q_ch(0, 3) + k_ch(1, 1) + k_ch(1, 2),
                (3, 0): lambda: k_ch(1, 3) + q_ch(1, 1),
                (0, 1): lambda: q_ch(1, 2) + q_ch(2, 0) + k_ch(2, 0),
                (1, 1): lambda: q_ch(1, 3) + k_ch(2, 1),
                (2, 1): lambda: k_ch(2, 2) + k_ch(2, 3),
                (3, 1): lambda: q_ch(2, 1),
                (0, 2): lambda: q_ch(2, 2),
                (1, 2): lambda: (q_ch(2, 3)
                                 + [op for ic in range(0, 4)
                                    for op in emit_proj(ic)]),
                (2, 2): lambda: [op for ic in range(4, 8)
                                 for op in emit_proj(ic)],
                (3, 2): lambda: [op for ic in range(8, 12)
                                 for op in emit_proj(ic)],
            }
            per_steps = {(0, 0): 11, (1, 0): 2, (2, 0): 2, (3, 0): 1,
                         (0, 1): 2, (1, 1): 1, (2, 1): 1, (3, 1): 1,
                         (0, 2): 1, (1, 2): 3, (2, 2): 2, (3, 2): 2}
            blocks = [(ib, hp) for hp in range(CCL) for ib in range(N // 512)]
            carried_et = None

            def emit_sx(ib, hp, j):
                st = psum.tile([128, 1024], f32, tag="st", bufs=2, name="st")
                for h2 in range(2):
                    hb = h2 * 64
                    nc.tensor.matmul(
                        st[:, h2 * 512:(h2 + 1) * 512],
                        lhsT=kt[hp][hb:hb + 64, j * 128:(j + 1) * 128],
                        rhs=qt[hp][hb:hb + 64, ib * 512:(ib + 1) * 512],
                        start=True, stop=True,
                    )
                et = work.tile([128, 1024], bf16, tag="et", bufs=8, name="et")
                nc.scalar.activation(et, st, Exp, scale=SCALE)
                return et

            for bi, (ib, hp) in enumerate(blocks):
                if (ib, hp) == (2, 2):
                    load.release()
                if (ib, hp) == (0, 0):
                    # weave: vt chunks stay ahead of their PV; k pair-0
                    # chunks m land before this block's step 4m; q0 chunk
                    # 1 is consumed by the very next block.
                    filler = []
                    vts = [emit_vt(nt) for nt in range(NJ)]
                    filler += vts[0] + k_ch(0, 1) + vts[1] + vts[2] + vts[3]
                    filler += k_ch(0, 2) + vts[4] + vts[5] + vts[6]
                    filler += k_ch(0, 3)
                    for v in vts[7:]:
                        filler += v
                    filler += q_ch(0, 1)
                else:
                    filler = fill_plan.get((ib, hp), lambda: [])()
                if (ib, hp) == (0, 0):
                    ramp = [3, 4, 5, 6, 8, 10, 14, 16,
                            16, 16, 16, 16, 16, 16, 16, 16]
                else:
                    ramp = [per_steps[(ib, hp)]] * NJ
                pv = [psum.tile([128, 512], f32, tag="pv", bufs=2,
                                name=f"pv{h2}") for h2 in range(2)]
                pv_q = []
                # last block: drain PVs eagerly so the tail after the final
                # exp is short (normalize + trailing proj start sooner)
                pvdepth = 1 if bi == len(blocks) - 1 else 4
                # scores are emitted in PAIRS of adjacent j so the row-tiled
                # score matmuls form one 4-MM group: the ~100ns LDW gaps on
                # entering/leaving a tiled group are paid per pair, not per j
                ets = [None] * (NJ + 1)
                if carried_et is not None:
                    ets[0], ets[1] = carried_et
                for j in range(NJ):           # one kv chunk per step
                    if ets[j] is None:
                        ets[j] = emit_sx(ib, hp, j)
                        if j + 1 < NJ:
                            ets[j + 1] = emit_sx(ib, hp, j + 1)
                    et = ets[j]
                    for _ in range(ramp[j]):
                        if filler:
                            filler.pop(0)()
                    if len(pv_q) >= pvdepth:
                        pv_q.pop(0)()

                    def mk_pv(j=j, et=et, pv=pv, hp=hp):
                        for h2 in range(2):
                            h = hp * 2 + h2
                            nc.tensor.matmul(
                                pv[h2],
                                lhsT=vt[j][:, h * 65:h * 65 + 128],
                                rhs=et[:, h2 * 512:(h2 + 1) * 512],
                                start=(j == 0), stop=(j == NJ - 1),
                            )
                    pv_q.append(mk_pv)
                # force-drain BEFORE the trailing PVs: leftover matmuls fill
                # the PE/ACT bubble while this block's final exps run.  The
                # next block's first TWO score groups are emitted here so the
                # exp stream crosses the boundary without a bubble.
                while filler:
                    filler.pop(0)()
                if bi + 1 < len(blocks):
                    nib, nhp = blocks[bi + 1]
                    carried_et = [emit_sx(nib, nhp, 0), emit_sx(nib, nhp, 1)]
                else:
                    carried_et = None
                for f in pv_q:
                    f()
                last = bi + 1 >= len(blocks)
                srow = work.tile([1, 1024], f32, tag="srow", bufs=2,
                                 name="srow")
                pvbs = []
                for h2 in range(2):
                    nc.vector.tensor_copy(srow[0:1, h2 * 512:(h2 + 1) * 512],
                                          pv[h2][64:65, :])
                    if not last:
                        # copy pv out so the next block's accumulation can
                        # reuse the psum slot without waiting the multiply
                        pvb = work.tile([64, 512], f32, tag="pvb", bufs=4,
                                        name="pvb")
                        nc.vector.tensor_copy(pvb, pv[h2][0:64, :])
                    else:
                        pvb = pv[h2][0:64, :]   # last block: read psum direct
                    pvbs.append(pvb)
                sinv = work.tile([1, 1024], f32, tag="sinv", bufs=2,
                                 name="sinv")
                for h2 in range(2):
                    nc.vector.reciprocal_approx_fast(
                        sinv[0:1, h2 * 512:(h2 + 1) * 512],
                        srow[0:1, h2 * 512:(h2 + 1) * 512])
                for h2 in range(2):
                    bc = work.tile([64, 512], f32, tag="bc", bufs=4,
                                   name="bc")
                    nc.gpsimd.partition_broadcast(
                        bc, sinv[0:1, h2 * 512:(h2 + 1) * 512])
                    nc.vector.tensor_mul(
                        att[hp][h2 * 64:h2 * 64 + 64, ib * 512:(ib + 1) * 512],
                        pvbs[h2],
                        bc,
                    )
            while filler:
                filler.pop(0)()
            # tail: proj for the last query block
            for ic in range(12, 16):
                for op in emit_proj(ic):
                    op()

    nc.compile()
    return nc


def _get_nc():
    if "nc" not in _NC_CACHE:
        _NC_CACHE["nc"] = _build()
    return _NC_CACHE["nc"]


def _ensure_ntff_hook():
    """The agent image's ``antenv`` lacks ``axon_hooks``; synthesize it so
    ``run_bass_kernel_spmd(trace=True)`` can capture NTFF profiles."""
    import types
    try:
        from antenv.axon_hooks import get_axon_ntff_profile_hook  # noqa: F401
        return
    except ImportError:
        pass
    import antenv
    from trn_agent_boot.trn_boot import _ntff_profile_via_ctypes
    hook = _ntff_profile_via_ctypes("/opt/axon/libaxon_pjrt.so")
    mod = types.ModuleType("antenv.axon_hooks")
    mod._hook = hook
    mod.get_axon_ntff_profile_hook = lambda: mod._hook

    def _set(h):
        mod._hook = h

    mod.set_axon_ntff_profile_hook = _set
    sys.modules["antenv.axon_hooks"] = mod
    antenv.axon_hooks = mod


def kernel(trace=False, **inputs):
    x = np.asarray(inputs["x"], np.float32)
    qkv_w = np.asarray(inputs["qkv_w"], np.float32)
    proj_w = np.asarray(inputs["proj_w"], np.float32)
    proj_b = np.asarray(inputs["proj_b"], np.float32)

    nc = _get_nc()

    xTb = np.ascontiguousarray(x.transpose(0, 2, 1)).astype(ml_dtypes.bfloat16)
    wqkvT = np.ascontiguousarray(qkv_w.T).astype(ml_dtypes.bfloat16)  # [768, 2304]
    wprojT = np.ascontiguousarray(proj_w.T).astype(ml_dtypes.bfloat16)  # [768, 768]
    bias = np.ascontiguousarray(proj_b.reshape(1, C)).astype(np.float32)
    zbias = np.zeros_like(bias)

    in_maps = []
    for c in range(NCORES):
        b, g = divmod(c, 2)
        cols = slice(g * CL, (g + 1) * CL)
        qg = wqkvT[:, 0:C][:, cols]
        kg = wqkvT[:, C:2 * C][:, cols]
        vg = wqkvT[:, 2 * C:3 * C][:, cols]
        # packed column order [q0|k0 || q1|k1|q2|k2 | v]: pair-0 q/k ride in
        # the pri tensor; the rest (896 cols) go in wqrest
        x3 = xTb[b].reshape(6, 128, N)
        pri = np.concatenate(
            [x3[:, :, 0:512],
             np.concatenate([qg[:, 0:128], kg[:, 0:128]],
                            axis=1).reshape(6, 128, 256),
             vg.reshape(6, 128, CL)], axis=2)
        xrest = np.ascontiguousarray(
            x3[:, :, 512:N].reshape(6, 128, 3, 512).transpose(2, 0, 1, 3))
        wqrest = np.concatenate(
            [np.concatenate([qg[:, hp * 128:(hp + 1) * 128],
                             kg[:, hp * 128:(hp + 1) * 128]], axis=1)
             for hp in range(1, CCL)], axis=1)
        in_maps.append({
            "pri": np.ascontiguousarray(pri),
            "xrest": xrest,
            "wqrest": np.ascontiguousarray(wqrest),
            "wprojT": np.ascontiguousarray(wprojT[g * CL:(g + 1) * CL, :]),
            "bias": bias if g == 0 else zbias,
        })

    from concourse import bass_utils
    if trace:
        _ensure_ntff_hook()
        bass_utils.upload_artifacts = lambda tmpdir: tmpdir
    res = bass_utils.run_bass_kernel_spmd(
        nc, in_maps, core_ids=list(range(NCORES)), trace=trace,
    )

    out = np.empty((B, N, C), np.float32)
    for b in range(B):
        out[b] = res.results[2 * b]["out"]
        out[b] += res.results[2 * b + 1]["out"]

    if trace:
        return out, res
    return out



# revision 65
# speedup vs baseline: 1.0184x; 1.0184x over previous
"""Multi-head attention (B=4, N=2048, C=768, H=12, Dh=64) on 8 TRN2 NeuronCores.

Sharding: head-parallel within each batch. Core (b, g) (g = core % 2) computes
q/k/v for heads 6g..6g+5 of batch b over the FULL 2048-token sequence -- no
duplicated K/V work between the pair -- runs attention for its 6 heads x 2048
queries, and produces a PARTIAL projection over its 384 channels. The host
sums the two partial (bf16) [2048, 768] outputs per batch (host-side
all-reduce; no device collectives). The bias rides in core g=0's input; core
g=1 receives zeros, keeping the program SPMD-identical.

Per-core inputs (partition dim first; all host-repacked so every priority
DMA descriptor reads contiguous DRAM):
  pri    [6, 128, 1152] bf16  per 128-row chunk: [x cols 0:512 | q0|k0 | v]
                              -- the complete dependency set of the first
                              q/k chains and all v chains
  xrest  [3, 6, 128, 512] bf16  x token-columns 512:2048 in (colchunk, cc)
  wqrest [768, 512]  bf16  [q1|k1|q2|k2] weight columns
  wprojT [384, 768]  bf16  proj_w.T rows for this core's 384 channels
  bias   [1, 768]    f32   real bias for g=0, zeros for g=1
  out    [2048, 768] bf16  partial projection (bf16 halves the drain)

Pipeline (~266us measured; evolved 371 -> 275 -> 266 via trace analysis;
TensorE ~99% occupied through the middle, ScalarE exp = 214us busy floor):
  - 12 attention blocks (3 local head pairs x 4 query i-blocks) in PAIR-MAJOR
    order so each pair's q/k chains are produced as filler inside the
    previous pair's exp-bound blocks.
  - per kv chunk: the two heads' score matmuls (K=64, lhsT base partitions
    0/64 -> auto row-tiled, they run CONCURRENTLY in the PE array) write a
    [128, 1024] PSUM tile; ONE 1024-wide exp per chunk on ScalarE (scale
    folds 1/sqrt(Dh); no max-subtraction, |S| <= ~10).
  - scores are emitted in adjacent-j pairs and the next block's first TWO
    score groups are emitted before this block's trailing PVs, so the exp
    stream crosses block boundaries without a bubble (the ~1.2us/transition
    ACT gaps this removes were worth ~13us).
  - the PV pair runs TWO steps behind its exp; PV stationary is a 128-col
    window into the packed V tile [v_h|1|v_{h+1}|1|...]; psum row 64
    accumulates the softmax denominator, rows 65..127 are garbage.
  - q/k/v/proj matmul chains are emitted as "filler" ops drained into the
    exp-bound steps at a per-block rate; every chain fully emits before the
    block that reads it. Blocks (0,0)-(3,0) are PE-capacity-bound (the 16 v
    chains must materialize inside block (0,0)); the ~12us of ACT idle there
    is the irreducible cost of this sharding.
  - input DMA: a dma trigger BLOCKS its engine queue until the transfer
    completes, so the scalar(ACT) ring carries exactly two early pri pieces
    and nothing else; priority pieces split across sync/scalar/gpsimd rings
    (~83GB/s each), bulk x columns ordered by consumption time. First real
    exp starts ~17-21us in (vs ~29us before), of which 7.5us is fixed NEFF
    startup.
  - a ~7us burst of FULL-WIDTH (K=128, M=128) matmuls at t=7us flips the
    PE HAM clock gate to 2.4GHz before the first chains run (they would
    otherwise run at 1.2GHz; quadrant-sized warmup tiles never trip the HAM
    busy detector).  CAUTION: an 80-matmul tiny-tile burst once tripped a
    chip-wide ~1.2x power-state throttle for the entire run.
  - normalize: denominator rows copied to SBUF (reciprocal_approx_fast
    reading PSUM directly returned garbage -- keep the copy), per-head
    reciprocal, GpSimd partition broadcast, multiply; pv psum copied out so
    the next block's accumulation is released quickly (last block reads psum
    direct and drains PVs eagerly, pvdepth=1).
  - tail: proj for the last 4 output rows borrows the st psum banks (free
    after the last exp) so consecutive row-blocks double-buffer instead of
    serializing on the 2-bank qk pool; their outputs drain as split DMAs
    across three rings.
"""

import sys

if "/opt/trn_rl_repo" not in sys.path:
    sys.path.insert(0, "/opt/trn_rl_repo")

import numpy as np
import ml_dtypes

B, N, C = 4, 2048, 768
H, Dh = 12, 64
HL = 6             # heads per core
CL = HL * Dh       # 384 local channels
SCALE = Dh ** -0.5
CCH = C // 128     # 6 contraction chunks (x/qkv input dim)
CCL = CL // 128    # 3 local head-pair chunks
NCORES = 8
VW = (HL - 1) * 65 + 128  # padded width of packed v tiles (453)
WARMUP = True

_NC_CACHE = {}


def _build():
    import concourse.bass as bass
    import concourse.tile as tile
    import concourse.mybir as mybir
    from concourse import bacc

    f32 = mybir.dt.float32
    bf16 = mybir.dt.bfloat16
    Exp = mybir.ActivationFunctionType.Exp

    nc = bacc.Bacc(
        "TRN2",
        target_bir_lowering=False,
        debug=False,
        enable_asserts=False,
        num_devices=NCORES,
    )

    # pri: per 128-row chunk, [x cols 0:512 | q0|k0 weights] packed
    # contiguously on the host so one descriptor covers the first q/k
    # chains' dependencies.  xrest: x cols 512:2048 in (colchunk, cc)
    # pieces, each contiguous.  wqrest: packed [q1|k1|q2|k2|v] columns.
    pri_d = nc.dram_tensor("pri", [CCH, 128, 1152], bf16,
                           kind="ExternalInput").ap()
    xrest_d = nc.dram_tensor("xrest", [3, CCH, 128, 512], bf16,
                             kind="ExternalInput").ap()
    wqrest = nc.dram_tensor("wqrest", [C, 512], bf16,
                            kind="ExternalInput").ap()
    wprojT = nc.dram_tensor("wprojT", [CL, C], bf16, kind="ExternalInput").ap()
    bias = nc.dram_tensor("bias", [1, C], f32, kind="ExternalInput").ap()
    out = nc.dram_tensor("out", [N, C], bf16, kind="ExternalOutput").ap()

    with tile.TileContext(nc) as tc:
        from contextlib import ExitStack

        with ExitStack() as ctx:
            singles = ctx.enter_context(tc.tile_pool(name="singles", bufs=1))
            psum = ctx.enter_context(tc.tile_pool(name="psum", bufs=1, space="PSUM"))
            work = ctx.enter_context(tc.tile_pool(name="work", bufs=4))

            # warm the ACT exp table set while the input DMAs run
            dummy = singles.tile([1, 8], f32, tag="dummy", name="dummy")
            nc.vector.memset(dummy, 0.0)
            dummy2 = singles.tile([1, 8], f32, tag="dummy2", name="dummy2")
            nc.scalar.activation(dummy2, dummy, Exp)

            # warm the PE HAM clock gate during the input-DMA window: ~3us
            # of back-to-back tiny matmuls flips the PE to 2.4GHz before the
            # first real chains run (they'd otherwise run at 1.2GHz)
            if WARMUP:
                # FULL-WIDTH (K=128, M=128) matmuls: quadrant-sized warmup
                # tiles never tripped the HAM busy detector (chains stayed at
                # 1.2GHz).  memset on gpsimd so the warmup isn't queued
                # behind the DVE's vt-ones memsets.
                warm = singles.tile([128, 224], bf16, tag="warm", name="warm")
                nc.gpsimd.memset(warm, 0.0)
                wps = psum.tile([128, 1024], f32, tag="st", bufs=2,
                                name="warmps")
                for i in range(38):
                    nc.tensor.matmul(wps[:, 0:224], lhsT=warm[:, 0:128],
                                     rhs=warm, start=(i == 0), stop=(i == 37))

            # ---- input DMAs --------------------------------------------
            # The first scores+exp need pri[cc] = [x cols 0:512 | q0|k0
            # weights]: 6 contiguous descriptors split over the sync and
            # scalar rings (~3 each), landing ~10.5us in (7.5us of that is
            # fixed NEFF startup).  The x-column tail streams in 512-col
            # pieces ordered by consumption time; the scalar (ACT) ring
            # stops issuing before the exp stream begins.
            load = tc.alloc_tile_pool(name="load", bufs=1)
            pri = [load.tile([128, 1152], bf16, tag=f"pri{i}", name=f"pri{i}")
                   for i in range(CCH)]
            xt = [load.tile([128, 1536], bf16, tag=f"xt{i}", name=f"xt{i}")
                  for i in range(CCH)]
            wqb = [load.tile([128, 512], bf16, tag=f"wqb{i}", name=f"wqb{i}")
                   for i in range(CCH)]
            wqv = [pri[i][:, 768:1152] for i in range(CCH)]
            # NOTE: a dma trigger BLOCKS its engine queue until the transfer
            # completes (ring flow control), so the scalar(ACT) ring gets
            # exactly ONE early piece -- anything more would sit in front of
            # the first exps.  Everything else rides sync/gpsimd.
            nc.sync.dma_start(out=pri[0], in_=pri_d[0])
            nc.scalar.dma_start(out=pri[2], in_=pri_d[2])
            nc.gpsimd.dma_start(out=pri[3], in_=pri_d[3])
            nc.sync.dma_start(out=pri[1], in_=pri_d[1])
            nc.scalar.dma_start(out=pri[4], in_=pri_d[4])
            nc.gpsimd.dma_start(out=pri[5], in_=pri_d[5])
            for i in range(CCH):
                nc.sync.dma_start(out=xt[i][:, 0:512], in_=xrest_d[0, i])
            for i in range(CCH):
                nc.gpsimd.dma_start(out=xt[i][:, 512:1024], in_=xrest_d[1, i])
            for i in range(3):
                nc.sync.dma_start(out=xt[i][:, 1024:1536], in_=xrest_d[2, i])
                nc.gpsimd.dma_start(out=xt[i + 3][:, 1024:1536],
                                    in_=xrest_d[2, i + 3])
            for i in range(CCH):
                nc.sync.dma_start(out=wqb[i],
                                  in_=wqrest[i * 128:(i + 1) * 128, 0:512])
            wp = []
            for i in range(CCL):
                t = singles.tile([128, C], bf16, tag=f"wp{i}", name=f"wp{i}")
                nc.sync.dma_start(out=t, in_=wprojT[i * 128:(i + 1) * 128, :])
                wp.append(t)
            bias_bc = singles.tile([128, C], f32, tag="bias", name="bias_bc")
            nc.sync.dma_start(
                out=bias_bc,
                in_=bass.AP(tensor=bias.tensor, offset=bias.offset,
                            ap=[[0, 128]] + list(bias.ap[1:])),
            )

            # x token-columns c0:c0+512 of row-chunk cc (pri holds 0:512)
            def xcol(cc, c0, w=512):
                if c0 + w <= 512:
                    return pri[cc][:, c0:c0 + w]
                return xt[cc][:, c0 - 512:c0 - 512 + w]

            # wq column lookup in the packed [q0|k0 || q1|k1|q2|k2 | v] layout
            def wq_qk(cc, base, hp):
                is_k = 1 if base else 0
                if hp == 0:
                    return pri[cc][:, 512 + is_k * 128:512 + (is_k + 1) * 128]
                c0 = (hp - 1) * 256 + is_k * 128
                return wqb[cc][:, c0:c0 + 128]

            # ---- qkv storage -------------------------------------------
            qt = [singles.tile([128, N], bf16, tag=f"qt{i}", name=f"qt{i}")
                  for i in range(CCL)]
            kt = [singles.tile([128, N], bf16, tag=f"kt{i}", name=f"kt{i}")
                  for i in range(CCL)]
            vt = [singles.tile([128, VW], bf16, tag=f"vt{i}", name=f"vt{i}")
                  for i in range(N // 128)]
            att = [singles.tile([128, N], bf16, tag=f"att{i}", name=f"att{i}")
                   for i in range(CCL)]

            # one q-or-k chain: 6 accumulating matmuls + a copy-out.
            # `order` lets the prefix chains accumulate in DMA-arrival order;
            # `split_copy` copies the first 128 columns separately so the
            # first score matmul (which only needs k cols 0:128) starts early.
            def emit_chain(dst, base, hp, nch, order=None, split_copy=False):
                ops = []
                ps = psum.tile([128, 512], f32, tag="qk", bufs=2,
                               name=f"ch{base}{hp}{nch}")
                ccs = list(order) if order else list(range(CCH))
                for i, cc in enumerate(ccs):
                    ops.append(lambda base=base, hp=hp, nch=nch, cc=cc, ps=ps,
                               st=(i == 0), sp=(i == CCH - 1): nc.tensor.matmul(
                        ps,
                        lhsT=wq_qk(cc, base, hp),
                        rhs=xcol(cc, nch * 512),
                        start=st, stop=sp,
                    ))
                c0 = nch * 512
                if split_copy:
                    ops.append(lambda: nc.vector.tensor_copy(
                        dst[hp][:, c0:c0 + 128], ps[:, 0:128]))
                    ops.append(lambda: nc.vector.tensor_copy(
                        dst[hp][:, c0 + 128:c0 + 512], ps[:, 128:512]))
                else:
                    ops.append(lambda: nc.vector.tensor_copy(
                        dst[hp][:, c0:c0 + 512], ps))
                return ops

            def q_ch(hp, n, order=None):
                return emit_chain(qt, 0, hp, n, order)

            def k_ch(hp, n, order=None):
                return emit_chain(kt, CL, hp, n, order)

            # partial proj for one 128-row output block (6 matmuls + add + dma).
            # Tail ics (>=12) borrow the st psum banks (free after the last
            # exp) so consecutive ics double-buffer instead of serializing on
            # the 2-bank qk pool, and their outputs drain as split DMAs on
            # three rings.
            def emit_proj(ic):
                ops = []
                tail = ic >= 12
                if tail and ic % 2 == 0:
                    t = psum.tile([128, 1024], f32, tag="st", bufs=2,
                                  name=f"pjt{ic}")
                    pjs = [(t[:, 0:512], 0, 512), (t[:, 512:768], 512, 256)]
                else:
                    pjs = [(psum.tile([128, 512], f32, tag="qk", bufs=2,
                                      name=f"pj{ic}_{d0}")[:, 0:dw], d0, dw)
                           for (d0, dw) in ((0, 512), (512, 256))]
                for cc in range(CCL):
                    for (pj, d0, dw) in pjs:
                        ops.append(lambda ic=ic, d0=d0, dw=dw, cc=cc, pj=pj: nc.tensor.matmul(
                            pj,
                            lhsT=att[cc][:, ic * 128:(ic + 1) * 128],
                            rhs=wp[cc][:, d0:d0 + dw],
                            start=(cc == 0), stop=(cc == CCL - 1),
                        ))
                def fin(ic=ic, pjs=pjs, tail=tail):
                    osb = work.tile([128, C], bf16, tag="osb", bufs=3,
                                    name=f"osb{ic}")
                    for (pj, d0, dw) in pjs:
                        nc.vector.tensor_add(osb[:, d0:d0 + dw], pj,
                                             bias_bc[:, d0:d0 + dw])
                    qs = [nc.sync, nc.gpsimd, nc.scalar]
                    if tail:
                        qa = qs[ic % 3]
                        qb = qs[(ic + 1) % 3]
                        qa.dma_start(out=out[ic * 128:ic * 128 + 64, :],
                                     in_=osb[0:64, :])
                        qb.dma_start(out=out[ic * 128 + 64:(ic + 1) * 128, :],
                                     in_=osb[64:128, :])
                    else:
                        qs[ic % 2].dma_start(out=out[ic * 128:(ic + 1) * 128, :],
                                             in_=osb)
                ops.append(fin)
                return ops

            # v in [token, d] layout, packed [v_h(64)|1] x 6 heads + pad.
            def emit_vt(nt):
                ops = []
                vaug = vt[nt][:, 0:HL * 65].rearrange("p (h e) -> p h e", e=65)
                ops.append(lambda vaug=vaug: nc.vector.memset(
                    vaug[:, :, 64:65], 1.0))
                ops.append(lambda nt=nt: nc.vector.memset(
                    vt[nt][:, HL * 65:VW], 0.0))
                ps = psum.tile([128, 512], f32, tag="qk", bufs=2,
                               name=f"psv{nt}")
                for cc in range(CCH):
                    ops.append(lambda nt=nt, cc=cc, ps=ps: nc.tensor.matmul(
                        ps[:, 0:CL],
                        lhsT=xcol(cc, nt * 128, 128),
                        rhs=wqv[cc],
                        start=(cc == 0), stop=(cc == CCH - 1),
                    ))
                ops.append(lambda vaug=vaug, ps=ps: nc.vector.tensor_copy(
                    vaug[:, :, 0:64],
                    ps[:, 0:CL].rearrange("p (h e) -> p h e", e=64),
                ))
                return ops

            # ---- phase 0: minimal prefix -------------------------------
            # accumulate in DMA-arrival order (pri pieces land one per ring);
            # v0 is NOT here -- its weights land later and would block the
            # first scores in the PE queue (it leads the block-0 weave)
            ARR = [2, 0, 3, 1, 5, 4]
            for op in q_ch(0, 0, order=ARR):
                op()
            for op in emit_chain(kt, CL, 0, 0, order=ARR, split_copy=True):
                op()

            # ---- attention ---------------------------------------------
            # per-block filler: matmul-ish ops interleaved into exp-bound
            # steps so the PE never idles while ScalarE runs exp.
            NJ = N // 128                     # 16 kv chunks
            filler = []
            # pair-major block order: each head pair's q/k chains are
            # produced inside the PREVIOUS pair's exp-bound blocks.
            fill_plan = {
                (1, 0): lambda: q_ch(0, 2) + q_ch(1, 0) + k_ch(1, 0),
                (2, 0): lambda: q_ch(0, 3) + k_ch(1, 1) + k_ch(1, 2),
                (3, 0): lambda: k_ch(1, 3) + q_ch(1, 1),
                (0, 1): lambda: q_ch(1, 2) + q_ch(2, 0) + k_ch(2, 0),
                (1, 1): lambda: q_ch(1, 3) + k_ch(2, 1),
                (2, 1): lambda: k_ch(2, 2) + k_ch(2, 3),
                (3, 1): lambda: q_ch(2, 1),
                (0, 2): lambda: q_ch(2, 2),
                (1, 2): lambda: (q_ch(2, 3)
                                 + [op for ic in range(0, 4)
                                    for op in emit_proj(ic)]),
                (2, 2): lambda: [op for ic in range(4, 8)
                                 for op in emit_proj(ic)],
                (3, 2): lambda: [op for ic in range(8, 12)
                                 for op in emit_proj(ic)],
            }
            per_steps = {(0, 0): 11, (1, 0): 2, (2, 0): 2, (3, 0): 1,
                         (0, 1): 2, (1, 1): 1, (2, 1): 1, (3, 1): 1,
                         (0, 2): 1, (1, 2): 3, (2, 2): 2, (3, 2): 2}
            blocks = [(ib, hp) for hp in range(CCL) for ib in range(N // 512)]
            carried_et = None

            def emit_sx(ib, hp, j):
                st = psum.tile([128, 1024], f32, tag="st", bufs=2, name="st")
                for h2 in range(2):
                    hb = h2 * 64
                    nc.tensor.matmul(
                        st[:, h2 * 512:(h2 + 1) * 512],
                        lhsT=kt[hp][hb:hb + 64, j * 128:(j + 1) * 128],
                        rhs=qt[hp][hb:hb + 64, ib * 512:(ib + 1) * 512],
                        start=True, stop=True,
                    )
                et = work.tile([128, 1024], bf16, tag="et", bufs=8, name="et")
                nc.scalar.activation(et, st, Exp, scale=SCALE)
                return et

            for bi, (ib, hp) in enumerate(blocks):
                if (ib, hp) == (2, 2):
                    load.release()
                if (ib, hp) == (0, 0):
                    # weave: vt chunks stay ahead of their PV; k pair-0
                    # chunks m land before this block's step 4m; q0 chunk
                    # 1 is consumed by the very next block.
                    filler = []
                    vts = [emit_vt(nt) for nt in range(NJ)]
                    filler += vts[0] + k_ch(0, 1) + vts[1] + vts[2] + vts[3]
                    filler += k_ch(0, 2) + vts[4] + vts[5] + vts[6]
                    filler += k_ch(0, 3)
                    for v in vts[7:]:
                        filler += v
                    filler += q_ch(0, 1)
                else:
                    filler = fill_plan.get((ib, hp), lambda: [])()
                if (ib, hp) == (0, 0):
                    ramp = [3, 4, 5, 6, 8, 10, 14, 16,
                            16, 16, 16, 16, 16, 16, 16, 16]
                else:
                    ramp = [per_steps[(ib, hp)]] * NJ
                pv = [psum.tile([128, 512], f32, tag="pv", bufs=2,
                                name=f"pv{h2}") for h2 in range(2)]
                pv_q = []
                # last block: drain PVs eagerly so the tail after the final
                # exp is short (normalize + trailing proj start sooner)
                pvdepth = 1 if bi == len(blocks) - 1 else 4
                # scores are emitted in PAIRS of adjacent j so the row-tiled
                # score matmuls form one 4-MM group: the ~100ns LDW gaps on
                # entering/leaving a tiled group are paid per pair, not per j
                ets = [None] * (NJ + 1)
                if carried_et is not None:
                    ets[0], ets[1] = carried_et
                for j in range(NJ):           # one kv chunk per step
                    if ets[j] is None:
                        ets[j] = emit_sx(ib, hp, j)
                        if j + 1 < NJ:
                            ets[j + 1] = emit_sx(ib, hp, j + 1)
                    et = ets[j]
                    for _ in range(ramp[j]):
                        if filler:
                            filler.pop(0)()
                    if len(pv_q) >= pvdepth:
                        pv_q.pop(0)()

                    def mk_pv(j=j, et=et, pv=pv, hp=hp):
                        for h2 in range(2):
                            h = hp * 2 + h2
                            nc.tensor.matmul(
                                pv[h2],
                                lhsT=vt[j][:, h * 65:h * 65 + 128],
                                rhs=et[:, h2 * 512:(h2 + 1) * 512],
                                start=(j == 0), stop=(j == NJ - 1),
                            )
                    pv_q.append(mk_pv)
                # force-drain BEFORE the trailing PVs: leftover matmuls fill
                # the PE/ACT bubble while this block's final exps run.  The
                # next block's first TWO score groups are emitted here so the
                # exp stream crosses the boundary without a bubble.
                while filler:
                    filler.pop(0)()
                if bi + 1 < len(blocks):
                    nib, nhp = blocks[bi + 1]
                    carried_et = [emit_sx(nib, nhp, 0), emit_sx(nib, nhp, 1)]
                else:
                    carried_et = None
                for f in pv_q:
                    f()
                last = bi + 1 >= len(blocks)
                srow = work.tile([1, 1024], f32, tag="srow", bufs=2,
                                 name="srow")
                pvbs = []
                for h2 in range(2):
                    nc.vector.tensor_copy(srow[0:1, h2 * 512:(h2 + 1) * 512],
                                          pv[h2][64:65, :])
                    if not last:
                        # copy pv out so the next block's accumulation can
                        # reuse the psum slot without waiting the multiply
                        pvb = work.tile([64, 512], f32, tag="pvb", bufs=4,
                                        name="pvb")
                        nc.vector.tensor_copy(pvb, pv[h2][0:64, :])
                    else:
                        pvb = pv[h2][0:64, :]   # last block: read psum direct
                    pvbs.append(pvb)
                sinv = work.tile([1, 1024], f32, tag="sinv", bufs=2,
                                 name="sinv")
                for h2 in range(2):
                    nc.vector.reciprocal_approx_fast(
                        sinv[0:1, h2 * 512:(h2 + 1) * 512],
                        srow[0:1, h2 * 512:(h2 + 1) * 512])
                for h2 in range(2):
                    bc = work.tile([64, 512], f32, tag="bc", bufs=4,
                                   name="bc")
                    nc.gpsimd.partition_broadcast(
                        bc, sinv[0:1, h2 * 512:(h2 + 1) * 512])
                    nc.vector.tensor_mul(
                        att[hp][h2 * 64:h2 * 64 + 64, ib * 512:(ib + 1) * 512],
                        pvbs[h2],
                        bc,
                    )
            while filler:
                filler.pop(0)()
            # tail: proj for the last query block
            for ic in range(12, 16):
                for op in emit_proj(ic):
                    op()

    nc.compile()
    return nc


def _get_nc():
    if "nc" not in _NC_CACHE:
        _NC_CACHE["nc"] = _build()
    return _NC_CACHE["nc"]


def _ensure_ntff_hook():
    """The agent image's ``antenv`` lacks ``axon_hooks``; synthesize it so
    ``run_bass_kernel_spmd(trace=True)`` can capture NTFF profiles."""
    import types
    try:
        from antenv.axon_hooks import get_axon_ntff_profile_hook  # noqa: F401
        return
    except ImportError:
        pass
    import antenv
    from trn_agent_boot.trn_boot import _ntff_profile_via_ctypes
    hook = _ntff_profile_via_ctypes("/opt/axon/libaxon_pjrt.so")
    mod = types.ModuleType("antenv.axon_hooks")
    mod._hook = hook
    mod.get_axon_ntff_profile_hook = lambda: mod._hook

    def _set(h):
        mod._hook = h

    mod.set_axon_ntff_profile_hook = _set
    sys.modules["antenv.axon_hooks"] = mod
    antenv.axon_hooks = mod


def kernel(trace=False, **inputs):
    x = np.asarray(inputs["x"], np.float32)
    qkv_w = np.asarray(inputs["qkv_w"], np.float32)
    proj_w = np.asarray(inputs["proj_w"], np.float32)
    proj_b = np.asarray(inputs["proj_b"], np.float32)

    nc = _get_nc()

    xTb = np.ascontiguousarray(x.transpose(0, 2, 1)).astype(ml_dtypes.bfloat16)
    wqkvT = np.ascontiguousarray(qkv_w.T).astype(ml_dtypes.bfloat16)  # [768, 2304]
    wprojT = np.ascontiguousarray(proj_w.T).astype(ml_dtypes.bfloat16)  # [768, 768]
    bias = np.ascontiguousarray(proj_b.reshape(1, C)).astype(np.float32)
    zbias = np.zeros_like(bias)

    in_maps = []
    for c in range(NCORES):
        b, g = divmod(c, 2)
        cols = slice(g * CL, (g + 1) * CL)
        qg = wqkvT[:, 0:C][:, cols]
        kg = wqkvT[:, C:2 * C][:, cols]
        vg = wqkvT[:, 2 * C:3 * C][:, cols]
        # packed column order [q0|k0 || q1|k1|q2|k2 | v]: pair-0 q/k ride in
        # the pri tensor; the rest (896 cols) go in wqrest
        x3 = xTb[b].reshape(6, 128, N)
        pri = np.concatenate(
            [x3[:, :, 0:512],
             np.concatenate([qg[:, 0:128], kg[:, 0:128]],
                            axis=1).reshape(6, 128, 256),
             vg.reshape(6, 128, CL)], axis=2)
        xrest = np.ascontiguousarray(
            x3[:, :, 512:N].reshape(6, 128, 3, 512).transpose(2, 0, 1, 3))
        wqrest = np.concatenate(
            [np.concatenate([qg[:, hp * 128:(hp + 1) * 128],
                             kg[:, hp * 128:(hp + 1) * 128]], axis=1)
             for hp in range(1, CCL)], axis=1)
        in_maps.append({
            "pri": np.ascontiguousarray(pri),
            "xrest": xrest,
            "wqrest": np.ascontiguousarray(wqrest),
            "wprojT": np.ascontiguousarray(wprojT[g * CL:(g + 1) * CL, :]),
            "bias": bias if g == 0 else zbias,
        })

    from concourse import bass_utils
    if trace:
        _ensure_ntff_hook()
        bass_utils.upload_artifacts = lambda tmpdir: tmpdir
    res = bass_utils.run_bass_kernel_spmd(
        nc, in_maps, core_ids=list(range(NCORES)), trace=trace,
    )

    out = np.empty((B, N, C), np.float32)
    for b in range(B):
        out[b] = res.results[2 * b]["out"]
        out[b] += res.results[2 * b + 1]["out"]

    if trace:
        return out, res
    return out

